# revision 5
# baseline (speedup 1.0000x reference)
"""Trainium2 Bass kernel for topk_masking IoU-accuracy reduction.

Problem: prob [262144, 392] f32, label [262144] int64 (values < 392).
reference = mean over rows of  inter/union  where pred = top-5 mask of the row
(strictly greater than the 6th-largest value), inter = pred[label],
union = |pred| + 1 - inter.

Math (exact for tie-free random f32 data):
  x   = prob[i, label[i]]
  hit = [ #(j : prob[i,j] >= x) <= 5 ]          (equivalent to x > 6th-largest)
  result = 0.2 * (#hits) / B                    (|pred| == 5 on tie-free data)

Key layout trick: the count #(prob[i,:] >= x) is invariant to any permutation
of the row's columns, so the host rotates every row left by label[i] when it
stages each core's shard.  The label element then sits at column 0 of every
row, and the per-block threshold x is just an AP slice of the streamed tile —
no on-device gather / mask-extraction pass at all.

Sharding: pure data parallel over the batch axis across 8 cores
(32768 rows/core).  Each core reduces to a [128,1] per-partition hit count;
the host sums 8x128 values and scales.  Rows are assigned to (partition,
block) so each partition reads one contiguous HBM run per superblock DMA:
  row(sb, p, b) = sb*128*DB + p*DB + b      (DB = blocks per superblock)

Per-core engine split (Bresenham-interleaved so both run concurrently):
  ScalarE: activation(Sign, scale=-1, bias=x), accum -> s
           (hit <=> s >= C-9.5)
  VectorE: tensor_scalar (P >= x per-partition-scalar), accum -> c
           (hit <=> c <= 5.5)
  epilogue: threshold both stat ranges, reduce-add -> acc [128,1], DMA out.
"""

import numpy as np

B = 262144
C = 392
NCORES = 8
RPC = B // NCORES          # rows per core
P = 128                    # SBUF partitions (rows per block)
K_TOP = 5                  # top-K; hit <=> #(P >= x) <= K_TOP
# sign-path threshold: hit <=> s >= 2*(C-K) - (C-1) - 0.5 = C - 9.5
S_THRESH = float(C) - 9.5

N_ACT = 113      # blocks counted on ScalarE (sign path); rest on VectorE
DMA_BLOCKS = 16  # 128-row blocks per prob dma_start (contig per partition)
TAIL_BLOCKS = 4  # the last superblock streams in chunks this wide
PBLK_BUFS = 5

_CACHE = {}
LAST_RESULTS = None


def _ensure_concourse():
    try:
        import concourse  # noqa: F401
    except ImportError:
        import sys
        if "/opt/trn_rl_repo" not in sys.path:
            sys.path.insert(0, "/opt/trn_rl_repo")


def emit_body(tc, prob_ap, out_ap, T, dma_blocks=DMA_BLOCKS, n_act=N_ACT,
              pblk_bufs=PBLK_BUFS):
    """Emit the per-core Tile program.

    prob_ap: [T*128, C] f32 DRAM, every row pre-rotated so x is column 0
    out_ap:  [128, 1]  f32 DRAM (per-partition hit counts)
    """
    from concourse import mybir

    nc = tc.nc
    f32 = mybir.dt.float32
    Alu = mybir.AluOpType
    Act = mybir.ActivationFunctionType

    assert T % dma_blocks == 0
    assert 0 <= n_act <= T
    n_super = T // dma_blocks
    n_dve = T - n_act

    with (
        tc.tile_pool(name="pblk", bufs=pblk_bufs) as pblk_pool,
        tc.tile_pool(name="junkc", bufs=2) as junkc_pool,
        tc.tile_pool(name="junks", bufs=2) as junks_pool,
        tc.tile_pool(name="stat", bufs=1) as stat_pool,
    ):
        # smat: sign-sums (ScalarE blocks); cmat: counts (VectorE blocks)
        smat = stat_pool.tile([P, max(n_act, 1)], f32)
        cmat = stat_pool.tile([P, max(n_dve, 1)], f32)

        # --- main loop ---
        # partition p reads dma_blocks consecutive rows per superblock
        prob3 = prob_ap.rearrange("(s p b) c -> s p (b c)", p=P, b=dma_blocks)
        sc = 0
        dc = 0
        for sb in range(n_super):
            ptile = pblk_pool.tile([P, dma_blocks * C], f32)
            nc.sync.dma_start(ptile[:], prob3[sb])
            for bb in range(dma_blocks):
                t = sb * dma_blocks + bb
                pblk = ptile[:, bb * C:(bb + 1) * C]
                xcol = ptile[:, bb * C:bb * C + 1]   # rotated: x == column 0

                if (t * n_act) % T < n_act:
                    junks = junks_pool.tile([P, C], f32)
                    # out = sign(x - P) ; accum_out = s
                    nc.scalar.activation(
                        junks[:],
                        pblk,
                        Act.Sign,
                        bias=xcol,
                        scale=-1.0,
                        accum_out=smat[:, sc:sc + 1],
                    )
                    sc += 1
                else:
                    junkc = junkc_pool.tile([P, C], f32)
                    # out = (P >= x) ; accum_out = reduce_add(out) = #(P >= x)
                    # (op1 is the reduction operator for TensorScalarPtrReduce)
                    nc.vector.tensor_scalar(
                        out=junkc[:],
                        in0=pblk,
                        scalar1=xcol,
                        scalar2=None,
                        op0=Alu.is_ge,
                        op1=Alu.add,
                        accum_out=cmat[:, dc:dc + 1],
                    )
                    dc += 1
        assert sc == n_act and dc == n_dve

        # --- epilogue: hits per partition ---
        hmat = stat_pool.tile([P, T], f32)
        if n_act > 0:
            nc.vector.tensor_scalar(
                out=hmat[:, :n_act], in0=smat[:, :n_act],
                scalar1=S_THRESH, scalar2=None, op0=Alu.is_ge,
            )
        if n_dve > 0:
            nc.vector.tensor_scalar(
                out=hmat[:, n_act:], in0=cmat[:, :n_dve],
                scalar1=float(K_TOP) + 0.5, scalar2=None, op0=Alu.is_le,
            )
        accs = stat_pool.tile([P, 1], f32)
        nc.vector.tensor_reduce(
            out=accs[:], in_=hmat[:], axis=mybir.AxisListType.X, op=Alu.add,
        )
        nc.sync.dma_start(out_ap, accs[:])


def build_program(rows_per_core=RPC, dma_blocks=DMA_BLOCKS, n_act=None,
                  pblk_bufs=PBLK_BUFS):
    _ensure_concourse()
    import concourse.tile as tile
    from concourse import bacc, mybir

    if n_act is None:
        n_act = N_ACT
    T = rows_per_core // P
    nc = bacc.Bacc(
        "TRN2",
        target_bir_lowering=False,
        debug=False,
        num_devices=NCORES,
    )
    prob = nc.dram_tensor(
        "prob", [rows_per_core, C], mybir.dt.float32, kind="ExternalInput"
    ).ap()
    out = nc.dram_tensor(
        "acc", [P, 1], mybir.dt.float32, kind="ExternalOutput"
    ).ap()
    with tile.TileContext(nc) as tc:
        emit_body(tc, prob, out, T, dma_blocks=dma_blocks, n_act=n_act,
                  pblk_bufs=pblk_bufs)
    nc.compile()
    return nc


def _rotate_shard(prob_shard, label_shard):
    """Rotate row i left by label[i] so the label element is at column 0."""
    lab = np.asarray(label_shard).astype(np.int32).reshape(-1, 1)
    cols = lab + np.arange(C, dtype=np.int32)[None, :]
    cols -= (cols >= C) * np.int32(C)
    return np.ascontiguousarray(
        np.take_along_axis(prob_shard, cols, axis=1))


def kernel(prob, label):
    global LAST_RESULTS
    _ensure_concourse()
    from concourse.bass_utils import run_bass_kernel_spmd

    prob = np.asarray(prob)
    label = np.asarray(label)
    assert prob.shape == (B, C) and label.shape == (B,)
    if prob.dtype != np.float32:
        prob = prob.astype(np.float32)

    if "nc" not in _CACHE:
        _CACHE["nc"] = build_program()
    nc = _CACHE["nc"]

    in_maps = []
    for ci in range(NCORES):
        in_maps.append({
            "prob": _rotate_shard(prob[ci * RPC:(ci + 1) * RPC],
                                  label[ci * RPC:(ci + 1) * RPC]),
        })

    res = run_bass_kernel_spmd(nc, in_maps, core_ids=list(range(NCORES)))
    LAST_RESULTS = res

    hits = 0.0
    for r in res.results:
        hits += float(np.asarray(r["acc"], dtype=np.float64).sum())
    return np.asarray(np.float32(0.2 * hits / B))
